# revision 7
# baseline (speedup 1.0000x reference)
"""Trainium2 Bass kernel for additive-relu attention (raw bass, explicit sync).

Reference computation (B=2, N=512, C=256):
    q, k, v = x @ Wq.T, x @ Wk.T, x @ Wv.T          # [B, N, C]
    score[b,i,j] = sum_d relu(q[b,i,d] + k[b,j,d])  # [B, N, N]
    attn = softmax(score, axis=-1)
    out = (attn @ v) @ Wp.T + bp

Sharding: data-parallel over (batch, query-block-of-128) -> 8 cores.  Each
core receives its batch's x ROTATED so its 128 queries are rows 0:128
(softmax and attn@v are invariant to a consistent key permutation), runs a
flash-style kernel over all 512 keys, and writes its [128, 256] output block.

Per-core dataflow:
  PRE : DMA x/W; PE-transpose -> xT (feature-on-partitions) and WT; project
        kT [d, keys], qT [d, queries], V [keys, d_v] on the PE.
  MAIN: per (query q, d-half h): R = relu(kT_h + qT_h[:, q]) elementwise on
        DVE (tensor_scalar add+max, 2x fp32 mode) and ACT (activation Relu
        with per-partition bias); d-reduction on the PE via matmul with a
        shifted one-hot-column ones matrix (float32r: full-rate) that
        accumulates score row q of S [128 queries, 512 keys] in PSUM.
  TAIL: softmax (reduce_max(negate) -> exp(bias=-max, accum_out)); 1/r is
        folded into a diagonal used as the rhs of the U-transpose; attn @ V
        and the output projection + bias on the PE; final transpose; DMA out.

Raw bass with explicit semaphores (Tile's auto-sync emits multi-wait
instructions this walrus rejects); every wait is a standalone instruction.
"""

import numpy as np

import concourse.bass as bass
import concourse.mybir as mybir
from concourse.bass_utils import run_bass_kernel_spmd

B, N, C = 2, 512, 256
P = 128
NCORES = 8
NR = 8                         # R ring slots
F32 = mybir.dt.float32
F32R = mybir.dt.float32r

AXT = mybir.ActivationFunctionType
ALU = mybir.AluOpType

NQH = 2 * P                    # (query, half) elementwise ops per core


def _use_dve(idx: int) -> bool:
    # DVE op ~327ns vs ACT ~613ns -> ~2/3 of ops on DVE
    return idx % 3 != 2


# producer rank tables: rank[i] = 1-based count of same-engine ops <= i
_DVE_RANK, _ACT_RANK = [], []
_d = _a = 0
for _i in range(NQH):
    if _use_dve(_i):
        _d += 1
    else:
        _a += 1
    _DVE_RANK.append(_d)
    _ACT_RANK.append(_a)
N_DVE_R, N_ACT_R = _d, _a


class EngState:
    """Tracks per-engine observed sem thresholds to elide covered waits."""

    def __init__(self, eng):
        self.eng = eng
        self.seen = {}

    def wait(self, sem, thr):
        if self.seen.get(sem.name, -1) >= thr:
            return
        self.eng.wait_ge(sem, thr)
        self.seen[sem.name] = thr


def _build_body(nc, xb, wq, wk, wv, wp, bp, ident_d, onesw_d, out_d):
    ident_h = nc.alloc_sbuf_tensor("ident_sb", [P, P], F32)
    ones_h = nc.alloc_sbuf_tensor("ones_shift", [P, 2 * P], F32R)
    xt_h = nc.alloc_sbuf_tensor("xt", [P, 4, C], F32)
    w_h = {n: nc.alloc_sbuf_tensor(f"w_{n}", [P, 2, C], F32) for n in "qkvp"}
    bpt_h = nc.alloc_sbuf_tensor("bpt", [P, 2], F32)
    xT_h = nc.alloc_sbuf_tensor("xT", [P, 2, N], F32)
    WT_h = {n: nc.alloc_sbuf_tensor(f"WT_{n}", [P, 2, C], F32) for n in "qkvp"}
    kT_h = nc.alloc_sbuf_tensor("kT", [P, 2, N], F32)
    qT_h = nc.alloc_sbuf_tensor("qT", [P, 2, P], F32)
    V_h = nc.alloc_sbuf_tensor("V", [P, 4, C], F32)
    R_h = nc.alloc_sbuf_tensor("R", [P, NR, N], F32R)
    U_h = nc.alloc_sbuf_tensor("U", [P, N], F32)
    Dm_h = nc.alloc_sbuf_tensor("Dm", [P, P], F32)
    attnT_h = nc.alloc_sbuf_tensor("attnT", [P, N], F32)
    OT_h = nc.alloc_sbuf_tensor("OT", [P, 2, P], F32)
    o2b_h = nc.alloc_sbuf_tensor("o2b", [P, 2, P], F32)
    fin_h = nc.alloc_sbuf_tensor("fin", [P, C], F32)
    negmx_h = nc.alloc_sbuf_tensor("negmx", [P, 1], F32)
    rsum_h = nc.alloc_sbuf_tensor("rsum", [P, 1], F32)
    rrec_h = nc.alloc_sbuf_tensor("rrec", [P, 1], F32)

    psA_h = nc.alloc_psum_tensor("psA", [P, N], F32)
    psB_h = nc.alloc_psum_tensor("psB", [P, N], F32)
    psS_h = nc.alloc_psum_tensor("psS", [P, N], F32)

    ident, ones, xt, bpt = ident_h.ap(), ones_h.ap(), xt_h.ap(), bpt_h.ap()
    wts = {n: h.ap() for n, h in w_h.items()}
    xT, kT, qT, V, R = xT_h.ap(), kT_h.ap(), qT_h.ap(), V_h.ap(), R_h.ap()
    WT = {n: h.ap() for n, h in WT_h.items()}
    U, Dm, attnT, OT = U_h.ap(), Dm_h.ap(), attnT_h.ap(), OT_h.ap()
    o2b, fin = o2b_h.ap(), fin_h.ap()
    negmx, rsum, rrec = negmx_h.ap(), rsum_h.ap(), rrec_h.ap()
    psA, psB, psS = psA_h.ap(), psB_h.ap(), psS_h.ap()

    # sem thresholds (computed up front)
    #   sD : DMA completions (x16)
    #   sPool: gpsimd const setup done
    #   sPE: one inc per PE result group (1..18 pre, 19..274 main, 275.. tail)
    #   sV : DVE op count, sA: ACT op count
    D_HEAD = 48                       # ident + onesw + x on sDh
    D_W = 64                          # all four weights on sDw
    PE_XT = [1, 2]
    PE_WT = {"k": [3, 4], "q": [5, 6], "v": [7, 8], "p": [9, 10]}
    PE_KT = [11, 12]
    PE_QT = [13, 14]
    PE_V = [15, 16, 17, 18]
    PE_MM0 = 19                       # main MM i -> PE_MM0 + i
    PE_ATT = PE_MM0 + NQH             # 275
    PE_OT = [PE_ATT + 1, PE_ATT + 2]
    PE_O2 = [PE_ATT + 3, PE_ATT + 4]
    PE_FIN = PE_ATT + 5

    # DVE stream: kT0,kT1,qT0,qT1,V0..V3 (1..8), R ops, negmx, rrec,
    #             OT0c, OT1c, o2b0, o2b1
    V_COPIES = 8
    V_NEGMX = V_COPIES + N_DVE_R + 1
    V_RREC = V_NEGMX + 1
    V_OTC = [V_RREC + 1, V_RREC + 2]
    V_O2B = [V_RREC + 3, V_RREC + 4]

    # ACT stream: xT0,xT1, WTk0..WTp1 (1..10), R ops, exp, Dm, attnTc, finc
    A_COPIES = 10
    A_EXP = A_COPIES + N_ACT_R + 1
    A_DM = A_EXP + 1
    A_ATTC = A_DM + 1
    A_FINC = A_ATTC + 1

    with (
        nc.semaphore("sDh") as sDh,
        nc.semaphore("sDw") as sDw,
        nc.semaphore("sDb") as sDb,
        nc.semaphore("sDo") as sDo,
        nc.semaphore("sPE") as sPE,
        nc.semaphore("sV") as sV,
        nc.semaphore("sA") as sA,
        nc.Block() as block,
    ):

        @block.sync
        def _(sync):
            sync.dma_start(out=ident, in_=ident_d).then_inc(sDh, 16)
            sync.dma_start(out=ones, in_=onesw_d).then_inc(sDh, 16)
            sync.dma_start(out=xt, in_=xb.rearrange("(t p) c -> p t c", p=P)
                           ).then_inc(sDh, 16)
            for name, w in (("k", wk), ("q", wq), ("v", wv), ("p", wp)):
                sync.dma_start(out=wts[name],
                               in_=w.rearrange("(t p) c -> p t c", p=P)
                               ).then_inc(sDw, 16)
            with nc.allow_non_contiguous_dma(reason="1KB bias load"):
                sync.dma_start(out=bpt, in_=bp.rearrange("(h p) -> p h", p=P)
                               ).then_inc(sDb, 16)
            sync.wait_ge(sA, A_FINC)
            sync.dma_start(out=out_d, in_=fin).then_inc(sDo, 16)
            sync.wait_ge(sDo, 16)

        @block.tensor
        def _(tensor):
            E = EngState(tensor)
            # pre: transposes into alternating psA/psB, one inc per group
            E.wait(sDh, D_HEAD)
            for h in range(2):          # xT
                ps = psA if h == 0 else psB
                for t in range(4):
                    mm = nc.tensor.transpose(
                        ps[:, t * P : (t + 1) * P],
                        xt[:, t, h * P : (h + 1) * P], ident)
                mm.then_inc(sPE, 1)
            E.wait(sDw, D_W)
            for name in "kqvp":
                for h in range(2):
                    ps = psA if h == 0 else psB
                    # WAR: previous reader of this psum slot (ACT copy)
                    E.wait(sA, PE_WT[name][h] - 2)
                    for t in range(2):
                        mm = nc.tensor.transpose(
                            ps[:, t * P : (t + 1) * P],
                            wts[name][:, t, h * P : (h + 1) * P], ident)
                    mm.then_inc(sPE, 1)
            # kT: needs all WT/xT copies; WAR on psA/psB from WTp copies
            E.wait(sA, A_COPIES)
            for h in range(2):
                ps = psA if h == 0 else psB
                for kc in range(2):
                    mm = nc.tensor.matmul(
                        ps, lhsT=WT["k"][:, kc, h * P : (h + 1) * P],
                        rhs=xT[:, kc, :], start=(kc == 0), stop=(kc == 1))
                mm.then_inc(sPE, 1)
            # qT: WAR vs DVE kT copies
            for h in range(2):
                ps = psA[:, 0:P] if h == 0 else psB[:, 0:P]
                E.wait(sV, 1 + h)
                for kc in range(2):
                    mm = nc.tensor.matmul(
                        ps, lhsT=WT["q"][:, kc, h * P : (h + 1) * P],
                        rhs=xT[:, kc, 0:P], start=(kc == 0), stop=(kc == 1))
                mm.then_inc(sPE, 1)
            # V: WAR vs DVE qT/V copies
            for jc in range(4):
                ps = psA[:, 0:C] if jc % 2 == 0 else psB[:, 0:C]
                E.wait(sV, 3 + jc)
                for kc in range(2):
                    mm = nc.tensor.matmul(
                        ps, lhsT=xT[:, kc, jc * P : (jc + 1) * P],
                        rhs=WT["v"][:, kc, :], start=(kc == 0), stop=(kc == 1))
                mm.then_inc(sPE, 1)
            # main: 256 accumulating one-hot reduction matmuls
            for i in range(NQH):
                q, h = divmod(i, 2)
                if _use_dve(i):
                    E.wait(sV, V_COPIES + _DVE_RANK[i])
                else:
                    E.wait(sA, A_COPIES + _ACT_RANK[i])
                nc.tensor.matmul(
                    psS,
                    lhsT=ones[:, P - q : 2 * P - q],
                    rhs=R[:, i % NR, :],
                    start=(i == 0),
                    stop=(i == NQH - 1),
                ).then_inc(sPE, 1)
            # attnT = U^T @ diag(1/r), into psB (WAR: V copies long done)
            E.wait(sA, A_DM)
            for t in range(4):
                mm = nc.tensor.matmul(
                    psB[:, t * P : (t + 1) * P],
                    lhsT=U[:, t * P : (t + 1) * P], rhs=Dm,
                    start=True, stop=True)
            mm.then_inc(sPE, 1)
            # OT[m] = (attn @ V).T halves, into psA
            E.wait(sA, A_ATTC)
            for m in range(2):
                if m == 1:
                    E.wait(sV, V_OTC[0])   # OT0 copied before opening OT1
                for jc in range(4):
                    mm = nc.tensor.matmul(
                        psA[:, m * P : (m + 1) * P],
                        lhsT=V[:, jc, m * P : (m + 1) * P],
                        rhs=attnT[:, jc * P : (jc + 1) * P],
                        start=(jc == 0), stop=(jc == 3))
                mm.then_inc(sPE, 1)
            # out2T halves = WpT @ OT, into psB
            for m in range(2):
                E.wait(sV, V_OTC[1] if m == 0 else V_O2B[0])
                for kc in range(2):
                    mm = nc.tensor.matmul(
                        psB[:, m * P : (m + 1) * P],
                        lhsT=WT["p"][:, kc, m * P : (m + 1) * P],
                        rhs=OT[:, kc, :], start=(kc == 0), stop=(kc == 1))
                mm.then_inc(sPE, 1)
            # final transpose [dp, i] -> [i, dp] into psA[:, 256:512]
            E.wait(sV, V_O2B[1])
            for m in range(2):
                mm = nc.tensor.transpose(
                    psA[:, C + m * P : C + (m + 1) * P], o2b[:, m, :], ident)
            mm.then_inc(sPE, 1)

        @block.vector
        def _(vector):
            E = EngState(vector)
            for h in range(2):          # kT copies
                E.wait(sPE, PE_KT[h])
                nc.vector.tensor_copy(kT[:, h, :], psA if h == 0 else psB
                                      ).then_inc(sV, 1)
            for h in range(2):          # qT copies
                E.wait(sPE, PE_QT[h])
                nc.vector.tensor_copy(
                    qT[:, h, :], (psA if h == 0 else psB)[:, 0:P]
                ).then_inc(sV, 1)
            for jc in range(4):         # V copies
                E.wait(sPE, PE_V[jc])
                nc.vector.tensor_copy(
                    V[:, jc, :], (psA if jc % 2 == 0 else psB)[:, 0:C]
                ).then_inc(sV, 1)
            for i in range(NQH):        # R (DVE share)
                if not _use_dve(i):
                    continue
                q, h = divmod(i, 2)
                if i >= NR:
                    E.wait(sPE, PE_MM0 + i - NR)
                nc.vector.tensor_scalar(
                    out=R[:, i % NR, :], in0=kT[:, h, :],
                    scalar1=qT[:, h, q : q + 1], scalar2=0.0,
                    op0=ALU.add, op1=ALU.max,
                ).then_inc(sV, 1)
            E.wait(sPE, PE_MM0 + NQH - 1)       # S complete
            nc.vector.tensor_reduce(
                out=negmx, in_=psS, axis=mybir.AxisListType.X,
                op=ALU.max, negate=True,
            ).then_inc(sV, 1)
            E.wait(sA, A_EXP)
            nc.vector.reciprocal(rrec, rsum).then_inc(sV, 1)
            for m in range(2):          # OT copies
                E.wait(sPE, PE_OT[m])
                nc.vector.tensor_copy(OT[:, m, :], psA[:, m * P : (m + 1) * P]
                                      ).then_inc(sV, 1)
            E.wait(sDb, 16)
            for m in range(2):          # out2T + bias -> sbuf
                E.wait(sPE, PE_O2[m])
                nc.vector.tensor_scalar(
                    out=o2b[:, m, :], in0=psB[:, m * P : (m + 1) * P],
                    scalar1=bpt[:, m : m + 1], scalar2=None, op0=ALU.add,
                ).then_inc(sV, 1)

        @block.scalar
        def _(scalar):
            E = EngState(scalar)
            for h in range(2):          # xT copies
                E.wait(sPE, PE_XT[h])
                nc.scalar.copy(xT[:, h, :], psA if h == 0 else psB
                               ).then_inc(sA, 1)
            for name in "kqvp":         # WT copies
                for h in range(2):
                    E.wait(sPE, PE_WT[name][h])
                    nc.scalar.copy(
                        WT[name][:, h, :], (psA if h == 0 else psB)[:, 0:C]
                    ).then_inc(sA, 1)
            E.wait(sV, 4)               # kT/qT written by DVE
            for i in range(NQH):        # R (ACT share)
                if _use_dve(i):
                    continue
                q, h = divmod(i, 2)
                if i >= NR:
                    E.wait(sPE, PE_MM0 + i - NR)
                nc.scalar.activation(
                    out=R[:, i % NR, :], in_=kT[:, h, :], func=AXT.Relu,
                    bias=qT[:, h, q : q + 1], scale=1.0,
                ).then_inc(sA, 1)
            E.wait(sPE, PE_MM0 + NQH - 1)
            E.wait(sV, V_NEGMX)
            nc.scalar.activation(
                out=U, in_=psS, func=AXT.Exp, bias=negmx, scale=1.0,
                accum_out=rsum,
            ).then_inc(sA, 1)
            E.wait(sV, V_RREC)
            nc.scalar.activation(
                out=Dm, in_=ident, func=AXT.Identity, bias=0.0, scale=rrec,
            ).then_inc(sA, 1)
            E.wait(sPE, PE_ATT)
            nc.scalar.copy(attnT, psB).then_inc(sA, 1)
            E.wait(sPE, PE_FIN)
            nc.scalar.copy(fin, psA[:, C : 2 * C]).then_inc(sA, 1)


_PROGRAM = None


def build_program():
    global _PROGRAM
    if _PROGRAM is not None:
        return _PROGRAM
    nc = bass.Bass(
        "TRN2", target_bir_lowering=False, debug=False, num_devices=NCORES
    )
    xb = nc.dram_tensor("xb", [N, C], F32, kind="ExternalInput")
    wq = nc.dram_tensor("wq", [C, C], F32, kind="ExternalInput")
    wk = nc.dram_tensor("wk", [C, C], F32, kind="ExternalInput")
    wv = nc.dram_tensor("wv", [C, C], F32, kind="ExternalInput")
    wp = nc.dram_tensor("wp", [C, C], F32, kind="ExternalInput")
    bp = nc.dram_tensor("bp", [C], F32, kind="ExternalInput")
    ident = nc.dram_tensor("ident", [P, P], F32, kind="ExternalInput")
    onesw = nc.dram_tensor("onesw", [P, 2 * P], F32R, kind="ExternalInput")
    out = nc.dram_tensor("out", [P, C], F32, kind="ExternalOutput")
    _build_body(nc, xb.ap(), wq.ap(), wk.ap(), wv.ap(), wp.ap(), bp.ap(),
                ident.ap(), onesw.ap(), out.ap())
    _PROGRAM = nc
    return nc


def make_in_maps(x, Wq, Wk, Wv, Wp, bp):
    """Per-core inputs: core = (batch, query-block); x rotated so the core's
    query block is rows 0:128."""
    x = np.ascontiguousarray(np.asarray(x, dtype=np.float32))
    onesw = np.zeros((P, 2 * P), dtype=np.float32)
    onesw[:, P] = 1.0
    common = {
        "ident": np.eye(P, dtype=np.float32),
        "onesw": onesw,
        "wq": np.ascontiguousarray(np.asarray(Wq, dtype=np.float32)),
        "wk": np.ascontiguousarray(np.asarray(Wk, dtype=np.float32)),
        "wv": np.ascontiguousarray(np.asarray(Wv, dtype=np.float32)),
        "wp": np.ascontiguousarray(np.asarray(Wp, dtype=np.float32)),
        "bp": np.ascontiguousarray(np.asarray(bp, dtype=np.float32)),
    }
    in_maps = []
    for core in range(NCORES):
        b, qb = divmod(core, NCORES // B)
        xrot = np.ascontiguousarray(np.roll(x[b], -qb * P, axis=0))
        in_maps.append({"xb": xrot, **common})
    return in_maps


def assemble(results):
    out = np.zeros((B, N, C), dtype=np.float32)
    for core in range(NCORES):
        b, qb = divmod(core, NCORES // B)
        out[b, qb * P : (qb + 1) * P] = results[core]["out"]
    return out


def kernel(x, Wq, Wk, Wv, Wp, bp):
    nc = build_program()
    in_maps = make_in_maps(x, Wq, Wk, Wv, Wp, bp)
    res = run_bass_kernel_spmd(nc, in_maps, list(range(NCORES)))
    return assemble(res.results)


if __name__ == "__main__":
    rng = np.random.default_rng(0)
    inputs = {
        "x": rng.standard_normal((B, N, C), dtype=np.float32),
        "Wq": rng.standard_normal((C, C), dtype=np.float32) * 0.02,
        "Wk": rng.standard_normal((C, C), dtype=np.float32) * 0.02,
        "Wv": rng.standard_normal((C, C), dtype=np.float32) * 0.02,
        "Wp": rng.standard_normal((C, C), dtype=np.float32) * 0.02,
        "bp": rng.standard_normal((C,), dtype=np.float32) * 0.02,
    }
    out = kernel(**inputs)
    print(out.shape, out.dtype)
